# revision 6
# baseline (speedup 1.0000x reference)
"""TRN2 Bass kernel for BLT self-attention (B=2, S=2048, HID=2048, 16 heads).

Sharding: 8 cores; core c owns batch b=c//4 and heads [(c%4)*4, (c%4)*4+4).
Each core computes a partial output (its 4 heads' contribution, [S, HID]);
host sums the 4 partials per batch.

Layout strategy (all heavy operands host-pre-transposed, fp32r matmuls):
  xt   [128,16,2048]  X^T chunks: xt[p,kc,s] = x[s, kc*128+p]
  wqt/wkt/wvt [128,16,512]: W^T chunks (q/k rows RoPE-pair-permuted)
  wot  [128,4,2048]: per-head Wo^T
  cosd/sind [128,2048]: duplicated/sign-adjusted rotary tables
  maskd [128,16,128]: diagonal 128x128 mask blocks, pre-divided by scale
Off-diagonal causal structure is exploited: fully-masked score tiles are
skipped (their softmax weight underflows to exactly 0 in the reference).
"""

import numpy as np

import concourse.bass as bass
import concourse.mybir as mybir
import concourse.tile as tile
from concourse import bacc
from concourse import bass_utils
from concourse.masks import make_identity

B, S, HID = 2, 2048, 2048
NH, DH = 16, 128
NCORES = 8
HPC = 4  # heads per core
NKC = HID // 128  # 16 contraction chunks
NQT = S // 128  # 16 q tiles
SCALE = DH ** -0.5
F32 = mybir.dt.float32
F32R = mybir.dt.float32r
AX = mybir.AxisListType.X
EXP = mybir.ActivationFunctionType.Exp

_NC_CACHE = None


def build():
    nc = bacc.Bacc("TRN2", target_bir_lowering=False, debug=False,
                   num_devices=NCORES)
    xt = nc.dram_tensor("xt", [128, NKC, S], F32R, kind="ExternalInput")
    wqt = nc.dram_tensor("wqt", [128, NKC, 512], F32R, kind="ExternalInput")
    wkt = nc.dram_tensor("wkt", [128, NKC, 512], F32R, kind="ExternalInput")
    wvt = nc.dram_tensor("wvt", [128, NKC, 512], F32R, kind="ExternalInput")
    wot = nc.dram_tensor("wot", [128, HPC, S], F32R, kind="ExternalInput")
    cosd = nc.dram_tensor("cosd", [128, S], F32, kind="ExternalInput")
    sind = nc.dram_tensor("sind", [128, S], F32, kind="ExternalInput")
    maskd = nc.dram_tensor("maskd", [128, NQT, 128], F32, kind="ExternalInput")
    out = nc.dram_tensor("out", [S, HID], F32, kind="ExternalOutput")

    with tile.TileContext(nc) as tc:
        with (
            tc.tile_pool(name="dram", bufs=1, space="DRAM") as dpool,
            tc.tile_pool(name="persist", bufs=1) as pp,
        ):
            qt_d = dpool.tile([128, HPC, S], F32R)
            v_d = dpool.tile([128, NKC, 512], F32R)
            kt_sb = pp.tile([128, HPC, S], F32R)
            ident = pp.tile([128, 128], F32)
            make_identity(nc, ident[:])
            maskd_sb = pp.tile([128, NQT, 128], F32)
            nc.sync.dma_start(maskd_sb[:], maskd.ap())
            zeros_sb = pp.tile([128, 128], F32)
            nc.gpsimd.memset(zeros_sb[:], 0.0)

            # ---------------- Phase 1: projections + rope ----------------
            with (
                tc.tile_pool(name="p1x", bufs=1) as xpool,
                tc.tile_pool(name="p1w", bufs=2) as wpool,
                tc.tile_pool(name="p1s", bufs=3) as spool,
                tc.tile_pool(name="p1cs", bufs=2) as cspool,
                tc.tile_pool(name="p1ps", bufs=4, space="PSUM") as ppsum,
                tc.tile_pool(name="p1tp", bufs=2, space="PSUM") as tpsum,
            ):
                for half in range(2):
                    s0 = half * 1024
                    xt_h = xpool.tile([128, NKC, 1024], F32R, tag="xt")
                    nc.sync.dma_start(xt_h[:], xt.ap()[:, :, s0:s0 + 1024])
                    cos_h = cspool.tile([128, 1024], F32, tag="cos")
                    sin_h = cspool.tile([128, 1024], F32, tag="sin")
                    nc.sync.dma_start(cos_h[:], cosd.ap()[:, s0:s0 + 1024])
                    nc.sync.dma_start(sin_h[:], sind.ap()[:, s0:s0 + 1024])
                    for h in range(HPC):
                        w_sb = {}
                        for pname, wt in (("q", wqt), ("k", wkt), ("v", wvt)):
                            w = wpool.tile([128, NKC, 128], F32R, tag=f"w{pname}")
                            nc.sync.dma_start(
                                w[:], wt.ap()[:, :, h * 128:(h + 1) * 128])
                            w_sb[pname] = w
                        for pname in ("q", "k", "v"):
                            for sb_i in range(2):
                                g = half * 2 + sb_i  # global 512-block of s
                                ps = ppsum.tile([128, 512], F32, tag="proj")
                                for kc in range(NKC):
                                    nc.tensor.matmul(
                                        ps[:], w_sb[pname][:, kc, :],
                                        xt_h[:, kc, sb_i * 512:(sb_i + 1) * 512],
                                        start=(kc == 0), stop=(kc == NKC - 1))
                                if pname in ("q", "k"):
                                    cs = cos_h[:, sb_i * 512:(sb_i + 1) * 512]
                                    sn = sin_h[:, sb_i * 512:(sb_i + 1) * 512]
                                    m1 = spool.tile([128, 512], F32, tag="m1")
                                    m2 = spool.tile([128, 512], F32, tag="m2")
                                    nc.vector.tensor_mul(m1[:], cs, ps[:])
                                    nc.vector.tensor_mul(
                                        m2[0:64, :], sn[0:64, :], ps[64:128, :])
                                    nc.vector.tensor_mul(
                                        m2[64:128, :], sn[64:128, :], ps[0:64, :])
                                    if pname == "k":
                                        nc.vector.tensor_add(
                                            kt_sb[:, h, g * 512:(g + 1) * 512],
                                            m1[:], m2[:])
                                    else:
                                        qsl = spool.tile([128, 512], F32R, tag="qsl")
                                        nc.vector.tensor_add(qsl[:], m1[:], m2[:])
                                        nc.sync.dma_start(
                                            qt_d[:, h, g * 512:(g + 1) * 512], qsl[:])
                                else:
                                    vt = spool.tile([128, 512], F32, tag="vt")
                                    nc.scalar.copy(vt[:], ps[:])
                                    for t in range(4):
                                        tp = tpsum.tile([128, 128], F32, tag="vtr")
                                        nc.tensor.transpose(
                                            tp[:], vt[:, t * 128:(t + 1) * 128],
                                            ident[:])
                                        vn = spool.tile([128, 128], F32R, tag="vn")
                                        nc.scalar.copy(vn[:], tp[:])
                                        nc.sync.dma_start(
                                            v_d[:, g * 4 + t, h * 128:(h + 1) * 128],
                                            vn[:])

            # ---------------- Phase 2 + 3 ----------------
            with tc.tile_pool(name="p23", bufs=1) as p23:
                attn_t = p23.tile([128, HPC, S], F32R)
                wot_sb = p23.tile([128, HPC, S], F32R)
                nc.sync.dma_start(wot_sb[:], wot.ap())

                with (
                    tc.tile_pool(name="p2qv", bufs=2) as qvpool,
                    tc.tile_pool(name="p2pb", bufs=2) as pbpool,
                    tc.tile_pool(name="p2pt", bufs=1) as ptpool,
                    tc.tile_pool(name="p2sm", bufs=4) as smpool,
                    tc.tile_pool(name="p2ps", bufs=5, space="PSUM") as s_ps,
                    tc.tile_pool(name="p2tp", bufs=2, space="PSUM") as t_psp,
                    tc.tile_pool(name="p2ap", bufs=1, space="PSUM") as a_psp,
                ):
                    pt_sb = ptpool.tile([128, NKC, 512], F32R)

                    for h in range(HPC):
                        qt_h = qvpool.tile([128, S], F32R, tag="qt")
                        nc.sync.dma_start(qt_h[:], qt_d[:, h, :])
                        v_h = qvpool.tile([128, NKC, 128], F32R, tag="vh")
                        nc.sync.dma_start(v_h[:], v_d[:, :, h * 128:(h + 1) * 128])

                        def post_stage(i, probs, h=h, qt_h=qt_h, v_h=v_h):
                            qoff = (i % 4) * 128
                            n_bt = (i + 4) // 4
                            for bt in range(n_bt):
                                nt = min(4, (i + 1) - bt * 4)
                                tp = t_psp.tile([128, 512], F32, tag="ptr")
                                for t in range(nt):
                                    kt = bt * 4 + t
                                    nc.tensor.transpose(
                                        tp[:, t * 128:(t + 1) * 128],
                                        probs[:, kt * 128:(kt + 1) * 128],
                                        ident[:])
                                dst = pt_sb[:, bt * 4:bt * 4 + nt, qoff:qoff + 128]
                                nc.scalar.copy(
                                    dst,
                                    tp[:, :nt * 128].rearrange(
                                        "p (k f) -> p k f", k=nt))
                            for kt in range(i + 1, 4 * (i // 4 + 1)):
                                nc.scalar.copy(
                                    pt_sb[:, kt, qoff:qoff + 128], zeros_sb[:])
                            if i % 4 == 3:
                                gg = i // 4
                                nkc = 4 * (gg + 1)
                                aps = a_psp.tile([128, 512], F32, tag="attn")
                                for kc in range(nkc):
                                    nc.tensor.matmul(
                                        aps[:], v_h[:, kc, :], pt_sb[:, kc, :],
                                        start=(kc == 0), stop=(kc == nkc - 1))
                                nc.scalar.copy(
                                    attn_t[:, h, gg * 512:(gg + 1) * 512], aps[:])

                        pending = None
                        for i in range(NQT):
                            nblk = i // 4 + 1
                            klen = (i + 1) * 128
                            probs = pbpool.tile([128, S], F32, tag="probs")
                            ps_blks = []
                            for blk in range(nblk):
                                kw = min(512, klen - blk * 512)
                                psb = s_ps.tile([128, 512], F32, tag="sc")
                                nc.tensor.matmul(
                                    psb[:, :kw], qt_h[:, i * 128:(i + 1) * 128],
                                    kt_sb[:, h, blk * 512:blk * 512 + kw],
                                    start=True, stop=True)
                                ps_blks.append((psb, kw))
                            db, off = i // 4, (i % 4) * 128
                            nc.vector.tensor_add(
                                ps_blks[db][0][:, off:off + 128],
                                ps_blks[db][0][:, off:off + 128],
                                maskd_sb[:, i, :])
                            mx = smpool.tile([128, 4], F32, tag="mx")
                            for blk, (psb, kw) in enumerate(ps_blks):
                                nc.vector.reduce_max(
                                    mx[:, blk:blk + 1], psb[:, :kw], axis=AX)
                            mxc = smpool.tile([128, 1], F32, tag="mxc")
                            nc.vector.reduce_max(mxc[:], mx[:, :nblk], axis=AX)
                            bias = smpool.tile([128, 1], F32, tag="bias")
                            nc.vector.tensor_scalar_mul(bias[:], mxc[:], -SCALE)
                            ls = smpool.tile([128, 4], F32, tag="ls")
                            for blk, (psb, kw) in enumerate(ps_blks):
                                nc.scalar.activation(
                                    probs[:, blk * 512:blk * 512 + kw],
                                    psb[:, :kw], EXP, bias=bias[:], scale=SCALE,
                                    accum_out=ls[:, blk:blk + 1])
                            lt = smpool.tile([128, 1], F32, tag="lt")
                            nc.vector.reduce_sum(lt[:], ls[:, :nblk], axis=AX)
                            linv = smpool.tile([128, 1], F32, tag="linv")
                            nc.vector.reciprocal(linv[:], lt[:])
                            nc.vector.tensor_scalar_mul(
                                probs[:, :klen], probs[:, :klen], linv[:])
                            if pending is not None:
                                post_stage(*pending)
                            pending = (i, probs)
                        post_stage(*pending)

                # ---------------- Phase 3: output projection ----------------
                with (
                    tc.tile_pool(name="p3ps", bufs=4, space="PSUM") as o_ps,
                    tc.tile_pool(name="p3st", bufs=4) as o_st,
                ):
                    for i in range(NQT):
                        for d in range(4):
                            ops_t = o_ps.tile([128, 512], F32, tag="o")
                            for h in range(HPC):
                                nc.tensor.matmul(
                                    ops_t[:], attn_t[:, h, i * 128:(i + 1) * 128],
                                    wot_sb[:, h, d * 512:(d + 1) * 512],
                                    start=(h == 0), stop=(h == HPC - 1))
                            ost = o_st.tile([128, 512], F32, tag="ost")
                            nc.scalar.copy(ost[:], ops_t[:])
                            nc.sync.dma_start(
                                out.ap()[i * 128:(i + 1) * 128,
                                         d * 512:(d + 1) * 512], ost[:])
    nc.compile()
    return nc


_PERM = np.concatenate([np.arange(64) * 2, np.arange(64) * 2 + 1])


def _chunked_T(w):
    # [R, C] -> [128, C//? ...]: w.T [C, R] -> [C//128, 128, R] -> [128, C//128, R]
    ct = np.ascontiguousarray(
        w.T.reshape(w.shape[1] // 128, 128, w.shape[0]).transpose(1, 0, 2))
    return ct


def _prep_core(c, hs, mask, cos, sin, wq, wk, wv, wo):
    b, h0 = c // HPC, (c % HPC) * HPC
    x = np.asarray(hs[b], np.float32)
    xt = _chunked_T(x)  # [128, 16, S]: xt[p,kc,s] = x[s, kc*128+p]

    def wmat(w, permute):
        rows = []
        for j in range(HPC):
            base = (h0 + j) * 128
            idx = base + (_PERM if permute else np.arange(128))
            rows.append(np.asarray(w, np.float32)[idx])
        w4 = np.concatenate(rows, 0)  # [512, HID]
        return _chunked_T(w4)  # [128, 16, 512]: [p,kc,f] = w4[f, kc*128+p]

    wqt = wmat(wq, True)
    wkt = wmat(wk, True)
    wvt = wmat(wv, False)
    wo_cols = np.asarray(wo, np.float32)[:, h0 * 128:(h0 + HPC) * 128]
    wot = _chunked_T(wo_cols)  # [128, 4, 2048]
    cb = np.asarray(cos[b], np.float32).T  # [64, S]
    sb_ = np.asarray(sin[b], np.float32).T
    cosd = np.ascontiguousarray(np.concatenate([cb, cb], 0))
    sind = np.ascontiguousarray(np.concatenate([-sb_, sb_], 0))
    m = np.asarray(mask[b, 0], np.float32)
    band = np.stack([m[i * 128:(i + 1) * 128, i * 128:(i + 1) * 128]
                     for i in range(NQT)])  # [16,128,128]
    maskd = np.ascontiguousarray(band.transpose(1, 0, 2)) * np.float32(1.0 / SCALE)
    return {"xt": xt, "wqt": wqt, "wkt": wkt, "wvt": wvt, "wot": wot,
            "cosd": cosd, "sind": sind, "maskd": maskd}


def _run(inputs):
    global _NC_CACHE
    if _NC_CACHE is None:
        _NC_CACHE = build()
    nc = _NC_CACHE
    in_maps = [
        _prep_core(c, inputs["hidden_states"], inputs["attention_mask"],
                   inputs["cos"], inputs["sin"], inputs["wq"], inputs["wk"],
                   inputs["wv"], inputs["wo"])
        for c in range(NCORES)
    ]
    return bass_utils.run_bass_kernel_spmd(nc, in_maps,
                                           core_ids=list(range(NCORES)))


def kernel(hidden_states, attention_mask, cos, sin, wq, wk, wv, wo):
    res = _run({"hidden_states": hidden_states, "attention_mask": attention_mask,
                "cos": cos, "sin": sin, "wq": wq, "wk": wk, "wv": wv, "wo": wo})
    out = np.zeros((B, S, HID), np.float32)
    for c in range(NCORES):
        out[c // HPC] += res.results[c]["out"]
    return out
